# revision 8
# baseline (speedup 1.0000x reference)
"""Trainium2 Bass kernel for nn_DynamicFiltering (v3).

Computation (per batch b):
  y  = LeakyReLU(conv2d(x_f, w1, b1), 0.2)      per frame f
  ker = conv2d(y, w2, b2)                        (9, h, w) per frame
  ker = ker - mean_k(ker) + 1/45                 (K = t*3*3 = 45 per pixel)
  out[c,h,w] = sum_{t,k1,k2} x_edge[c,t,h+k1-1,w+k2-1] * ker[t,k1,k2][h,w]

Sharding: 8 cores = 2 batches x 4 H-slabs of 32 rows.

Design notes:
  - fp16 on-chip throughout (gate is 2e-2, fp16 keeps ~15x margin)
  - convs with contraction packed to 128: partitions 64..127 hold a
    row-shifted (xs/ys) or row+col-shifted (xs01/ys01) copy, so the 9-tap
    conv runs as 5 matmuls per 4-row chunk (3 row-pairs + 1 col-pair + 1
    single); the shifted copies are built by SBUF->SBUF DMA
  - LeakyReLU via the hardware Prelu activation (alpha=0.2)
  - ker (9,r,w) -> (w,r,9) pivot via one XBAR dma_start_transpose per frame
  - filtering layout [pix, c, r] (r contiguous) for DVE 16-bit mode; the
    tap products come from DVE tensor_tensor, the accumulations run as
    gpsimd software-DGE DMA chains with accum_op=add (one chain per dj),
    keeping the DVE free for the multiplies
  - final merge: partition-shift the dj accumulators by DMA, add, then one
    XBAR transpose of [w, (c r)]; out is fp16 and the host converts
"""

import numpy as np

DIM = 64
T = 5
H = 128
W = 128
SLAB = 32          # output rows per core
NCORES = 8
GH = 36            # conv grid rows: slab + 2*2 halo
GW = 130           # conv grid cols: W + 2
FR = 34            # filter rows: slab + 2 halo

USE_QCONV = True
USE_DMA_ACCUM = False
GP_TAPS = ((0, 1), (1, 1), (2, 1))
_PROGRAM_CACHE = {}


def _build_program():
    import concourse.bacc as bacc
    import concourse.mybir as mybir
    from concourse.tile import TileContext

    f32 = mybir.dt.float32
    f16 = mybir.dt.float16
    Act = mybir.ActivationFunctionType
    Alu = mybir.AluOpType

    nc = bacc.Bacc("TRN2", debug=False)

    xs_d = nc.dram_tensor("xs", [128, T, GH, GW], f16, kind="ExternalInput").ap()
    xt_d = nc.dram_tensor("xt", [W, T, DIM, FR], f16, kind="ExternalInput").ap()
    w1s_d = nc.dram_tensor("w1s", [128, 3, DIM], f16, kind="ExternalInput").ap()
    w1r_d = nc.dram_tensor("w1r", [128, 3, DIM], f16, kind="ExternalInput").ap()
    w1q_d = nc.dram_tensor("w1q", [128, DIM], f16, kind="ExternalInput").ap()
    w2s_d = nc.dram_tensor("w2s", [128, 3, 9], f16, kind="ExternalInput").ap()
    w2r_d = nc.dram_tensor("w2r", [128, 3, 9], f16, kind="ExternalInput").ap()
    w2q_d = nc.dram_tensor("w2q", [128, 9], f16, kind="ExternalInput").ap()
    b1_d = nc.dram_tensor("b1c", [DIM, 1], f32, kind="ExternalInput").ap()
    b2_d = nc.dram_tensor("b2c", [9, 1], f32, kind="ExternalInput").ap()
    ym_d = nc.dram_tensor("ymask", [DIM, 2], f32, kind="ExternalInput").ap()
    em_d = nc.dram_tensor("emask", [W, 1], f32, kind="ExternalInput").ap()
    ef_d = nc.dram_tensor("efold", [W, 1], f32, kind="ExternalInput").ap()
    ea_d = nc.dram_tensor("emA", [W, 1], f32, kind="ExternalInput").ap()
    eb_d = nc.dram_tensor("emB", [W, 1], f32, kind="ExternalInput").ap()
    out_d = nc.dram_tensor("out", [DIM, SLAB, W], f16, kind="ExternalOutput").ap()

    with nc.allow_low_precision(reason="2e-2 gate; fp16 has ~15x margin"), \
            TileContext(nc) as tc:
        with (
            tc.tile_pool(name="consts", bufs=1) as cpool,
            tc.tile_pool(name="xsp", bufs=3) as xsp,
            tc.tile_pool(name="xqp", bufs=2) as xqp,
            tc.tile_pool(name="ysp", bufs=2) as ysp,
            tc.tile_pool(name="yqp", bufs=2) as yqp,
            tc.tile_pool(name="kerp", bufs=2) as kerp,
            tc.tile_pool(name="kt1p", bufs=2) as kt1p,
            tc.tile_pool(name="ktp", bufs=1) as ktp,
            tc.tile_pool(name="accp", bufs=1) as accp,
            tc.tile_pool(name="stage", bufs=4) as stp,
            tc.tile_pool(name="gstage", bufs=2) as gstp,
        ):
            # --- loads needed by conv1(0) go first on the SP queue
            xs_f_tiles = {}

            def load_xs(f):
                t = xsp.tile([128, GH, GW], f16, tag="xs")
                nc.sync.dma_start(out=t, in_=xs_d[:, f])
                xs_f_tiles[f] = t

            load_xs(0)
            w1s_sb = cpool.tile([128, 3, DIM], f16)
            nc.sync.dma_start(out=w1s_sb, in_=w1s_d)
            w1r_sb = cpool.tile([128, 3, DIM], f16)
            nc.sync.dma_start(out=w1r_sb, in_=w1r_d)
            w1q_sb = cpool.tile([128, DIM], f16)
            nc.sync.dma_start(out=w1q_sb, in_=w1q_d)
            b1_sb = cpool.tile([DIM, 1], f32)
            nc.sync.dma_start(out=b1_sb, in_=b1_d)
            load_xs(1)
            w2s_sb = cpool.tile([128, 3, 9], f16)
            nc.sync.dma_start(out=w2s_sb, in_=w2s_d)
            w2r_sb = cpool.tile([128, 3, 9], f16)
            nc.sync.dma_start(out=w2r_sb, in_=w2r_d)
            w2q_sb = cpool.tile([128, 9], f16)
            nc.sync.dma_start(out=w2q_sb, in_=w2q_d)
            b2_sb = cpool.tile([9, 1], f32)
            nc.sync.dma_start(out=b2_sb, in_=b2_d)
            ym_sb = cpool.tile([DIM, 2], f32)
            nc.sync.dma_start(out=ym_sb, in_=ym_d)

            # filter input frames (consumed from post(0) on)
            xt_sb = cpool.tile([W, T, DIM, FR], f16)
            for f in range(T):
                nc.sync.dma_start(out=xt_sb[:, f], in_=xt_d[:, f])

            em_sb = cpool.tile([W, 1], f32)
            nc.sync.dma_start(out=em_sb, in_=em_d)
            ef_sb = cpool.tile([W, 1], f32)
            nc.sync.dma_start(out=ef_sb, in_=ef_d)
            ea_sb = cpool.tile([W, 1], f32)
            nc.sync.dma_start(out=ea_sb, in_=ea_d)
            eb_sb = cpool.tile([W, 1], f32)
            nc.sync.dma_start(out=eb_sb, in_=eb_d)

            # per-pixel kernels, pixel-partitioned: kt[p, 9f+3di+dj, r]
            kt = ktp.tile([W, 48, SLAB], f16)
            ktr = kt[:, 0:45, :].rearrange(
                "p (t di dj) r -> p t di dj r", t=T, di=3, dj=3)
            kt_p1 = ktp.tile([W, 48, SLAB], f16)   # kt_p1[q] = kt[q+1]
            kt_m1 = ktp.tile([W, 48, SLAB], f16)   # kt_m1[q] = kt[q-1]
            nc.vector.memset(kt_p1[96:128], 0.0)
            nc.vector.memset(kt_m1[0:32], 0.0)

            # dj-separated accumulators (fp16), filled by DMA-accum chains
            accs = []
            for dj in range(3):
                a = accp.tile([W, DIM, SLAB], f16, name=f"acc{dj}")
                nc.vector.memset(a, 0.0)
                accs.append(a)
            u_sb = accp.tile([W, DIM, FR], f16)
            nc.vector.memset(u_sb, 0.0)
            gacc = accp.tile([W, DIM, SLAB], f16)
            nc.gpsimd.memset(gacc, 0.0)

            ys_tiles = {}
            ker_tiles = {}

            with (
                tc.tile_pool(name="ps1", bufs=4, space="PSUM") as ps1p,
                tc.tile_pool(name="ps2", bufs=4, space="PSUM") as ps2p,
            ):
                def conv1(f):
                    xs_f = xs_f_tiles[f]
                    # col-pair-stacked copy for taps (di=2, dj=0/1):
                    # xq[c+64a, g, w] = xpad[c, r0-2+g+2, w-1+a]
                    xq = xqp.tile([128, GH - 2, W], f16, tag="xq")
                    nc.sync.dma_start(out=xq[0:64], in_=xs_f[0:64, 2:36, 0:128])
                    nc.sync.dma_start(out=xq[64:128],
                                      in_=xs_f[64:128, 1:35, 1:129])
                    ys = ysp.tile([128, GH, GW], f16, tag="ys")
                    ys_tiles[f] = ys
                    nc.vector.memset(ys[0:64, 1:35, 0:1], 0.0)
                    nc.vector.memset(ys[0:64, 1:35, 129:130], 0.0)
                    for rc in range(9):
                        g0 = 1 + 4 * rc
                        nr = 4 if rc < 8 else 2
                        ps = ps1p.tile([DIM, 4, W], f32, tag="ps1")
                        for dj in range(3):
                            nc.tensor.matmul(
                                ps[:, :nr, :],
                                lhsT=w1s_sb[:, dj, :],
                                rhs=xs_f[:, g0 - 1:g0 - 1 + nr, dj:dj + W],
                                start=(dj == 0), stop=False)
                        if USE_QCONV:
                            nc.tensor.matmul(
                                ps[:, :nr, :], lhsT=w1q_sb,
                                rhs=xq[:, g0 - 1:g0 - 1 + nr, :],
                                start=False, stop=False)
                            nc.tensor.matmul(
                                ps[:, :nr, :], lhsT=w1r_sb[64:128, 2, :],
                                rhs=xs_f[64:128, g0:g0 + nr, 2:2 + W],
                                start=False, stop=True)
                        else:
                            for dj in range(3):
                                nc.tensor.matmul(
                                    ps[:, :nr, :],
                                    lhsT=w1r_sb[64:128, dj, :],
                                    rhs=xs_f[64:128, g0:g0 + nr, dj:dj + W],
                                    start=False, stop=(dj == 2))
                        nc.scalar.activation(
                            ys[0:64, g0:g0 + nr, 1:129], ps[:, :nr],
                            Act.Prelu, bias=b1_sb, scale=1.0, alpha=0.2)
                    # zero y halo rows outside the image (conv2 zero-pad)
                    nc.scalar.activation(ys[0:64, 1:2, 1:129],
                                         ys[0:64, 1:2, 1:129],
                                         Act.Copy, scale=ym_sb[:, 0:1])
                    nc.scalar.activation(ys[0:64, 34:35, 1:129],
                                         ys[0:64, 34:35, 1:129],
                                         Act.Copy, scale=ym_sb[:, 1:2])
                    # stacked row-shifted copy: ys[64+c, g] = ys[c, g+1]
                    nc.sync.dma_start(out=ys[64:128, 1:34, :],
                                      in_=ys[0:64, 2:35, :])

                def conv2(f):
                    ys = ys_tiles[f]
                    # col-pair-stacked copy for taps (di=2, dj=0/1):
                    # yq[o+64a, g, w] = y[o, r0-2+g+2, w-1+a]
                    yq = yqp.tile([128, 32, W], f16, tag="yq")
                    nc.sync.dma_start(out=yq[0:64], in_=ys[0:64, 3:35, 0:128])
                    nc.sync.dma_start(out=yq[64:128],
                                      in_=ys[0:64, 3:35, 1:129])
                    ker_f = kerp.tile([16, SLAB, W], f16, tag="ker")
                    ker_tiles[f] = ker_f
                    for rc in range(8):
                        g0 = 2 + 4 * rc
                        ps2 = ps2p.tile([9, 4, W], f32, tag="ps2")
                        for dj in range(3):
                            nc.tensor.matmul(
                                ps2,
                                lhsT=w2s_sb[:, dj, :],
                                rhs=ys[:, g0 - 1:g0 + 3, dj:dj + W],
                                start=(dj == 0), stop=False)
                        if USE_QCONV:
                            nc.tensor.matmul(
                                ps2, lhsT=w2q_sb,
                                rhs=yq[:, g0 - 2:g0 + 2, :],
                                start=False, stop=False)
                            nc.tensor.matmul(
                                ps2, lhsT=w2r_sb[64:128, 2, :],
                                rhs=ys[64:128, g0:g0 + 4, 2:2 + W],
                                start=False, stop=True)
                        else:
                            for dj in range(3):
                                nc.tensor.matmul(
                                    ps2, lhsT=w2r_sb[64:128, dj, :],
                                    rhs=ys[64:128, g0:g0 + 4, dj:dj + W],
                                    start=False, stop=(dj == 2))
                        nc.scalar.activation(
                            ker_f[0:9, 4 * rc:4 * rc + 4, :], ps2,
                            Act.Identity, bias=b2_sb, scale=1.0)

                def post(f):
                    # pivot ker (9, r, w) -> (w, r, 9) via the XBAR
                    ker_f = ker_tiles[f]
                    kt1 = kt1p.tile([W, SLAB, 16], f16, tag="kt1")
                    nc.sync.dma_start(out=kt1, in_=ker_f, transpose=True)
                    # repack to (w, 9, r): taps outer, rows contiguous
                    nc.scalar.copy(
                        kt[:, 9 * f:9 * f + 9, :],
                        kt1[:, :, 0:9].rearrange("p r k -> p k r"))
                    # fold W-edge replicate-pad terms into the dj=1 slot
                    nc.vector.tensor_tensor(ktr[0:1, f, :, 1, :],
                                            ktr[0:1, f, :, 1, :],
                                            ktr[0:1, f, :, 0, :], Alu.add)
                    nc.vector.scalar_tensor_tensor(
                        out=ktr[96:128, f, :, 1, :],
                        in0=ktr[96:128, f, :, 2, :], scalar=em_sb[96:128, :],
                        in1=ktr[96:128, f, :, 1, :],
                        op0=Alu.mult, op1=Alu.add)
                    # partition-shifted kernel copies for dj=0 / dj=2 taps
                    nc.sync.dma_start(out=kt_p1[0:127, 9 * f:9 * f + 9, :],
                                      in_=kt[1:128, 9 * f:9 * f + 9, :])
                    nc.sync.dma_start(out=kt_m1[1:128, 9 * f:9 * f + 9, :],
                                      in_=kt[0:127, 9 * f:9 * f + 9, :])
                    # dynamic filtering: DVE multiplies, DMA-accum adds
                    ksrc = [kt_p1, kt, kt_m1]
                    for di in range(3):
                        for dj in range(3):
                            kb = ksrc[dj][:, 9 * f + 3 * di + dj, :]\
                                .unsqueeze(1).broadcast_to((W, DIM, SLAB))
                            xt_sl = xt_sb[:, f, :, di:di + SLAB]
                            if (di, dj) in GP_TAPS:
                                prod = gstp.tile([W, DIM, SLAB], f16,
                                                 tag="gprod")
                                nc.gpsimd.tensor_tensor(prod, xt_sl, kb,
                                                        Alu.mult)
                                nc.gpsimd.tensor_tensor(gacc, gacc, prod,
                                                        Alu.add)
                            else:
                                prod = stp.tile([W, DIM, SLAB], f16,
                                                tag="prod")
                                nc.vector.tensor_tensor(prod, xt_sl, kb,
                                                        Alu.mult)
                                nc.vector.tensor_tensor(accs[dj], accs[dj],
                                                        prod, Alu.add)
                    # u += xt_f (for the normalization term c * S)
                    nc.vector.tensor_tensor(u_sb, u_sb, xt_sb[:, f],
                                            Alu.add)

                conv1(0)
                conv2(0)
                load_xs(2)
                post(0)
                conv1(1)
                conv2(1)
                load_xs(3)
                post(1)
                conv1(2)
                conv2(2)
                load_xs(4)
                post(2)
                conv1(3)
                conv2(3)
                post(3)
                conv1(4)
                conv2(4)
                post(4)

            # normalization: out += c * S with c = 1/45 - mean(ker);
            # sum45 reads the folded kernel, undo the edge double-count
            sum45 = ktp.tile([W, SLAB], f16)
            kt_v = kt[:, 0:45, :].rearrange("p (t n) r -> p r t n", t=T)
            nc.vector.tensor_reduce(sum45, kt_v, axis=mybir.AxisListType.XY,
                                    op=Alu.add)
            c_sb = ktp.tile([W, SLAB], f16)
            nc.vector.tensor_scalar(c_sb, sum45, -1.0 / 45.0, 1.0 / 45.0,
                                    Alu.mult, Alu.add)
            corr = ktp.tile([W, SLAB], f16)
            kt_e = kt[:, 0:45, :].rearrange(
                "p (t di dj) r -> p r t di dj", t=T, di=3, dj=3)
            nc.vector.tensor_reduce(corr[0:32], kt_e[0:32, :, :, :, 0],
                                    axis=mybir.AxisListType.XY, op=Alu.add)
            nc.vector.tensor_reduce(corr[96:128], kt_e[96:128, :, :, :, 2],
                                    axis=mybir.AxisListType.XY, op=Alu.add)
            nc.vector.scalar_tensor_tensor(out=c_sb[0:32], in0=corr[0:32],
                                           scalar=ea_sb[0:32], in1=c_sb[0:32],
                                           op0=Alu.mult, op1=Alu.add)
            nc.vector.scalar_tensor_tensor(out=c_sb[96:128], in0=corr[96:128],
                                           scalar=eb_sb[96:128],
                                           in1=c_sb[96:128],
                                           op0=Alu.mult, op1=Alu.add)

            # S = 3-row vertical box of u (edge rows already clamped in xt)
            s_sb = accp.tile([W, DIM, SLAB], f16)
            nc.vector.tensor_tensor(s_sb, u_sb[:, :, 0:SLAB],
                                    u_sb[:, :, 1:SLAB + 1], Alu.add)
            nc.vector.tensor_tensor(s_sb, s_sb, u_sb[:, :, 2:SLAB + 2],
                                    Alu.add)

            # shifted + edge-doubled variants of c
            c_p1 = ktp.tile([W, SLAB], f16)
            c_m1 = ktp.tile([W, SLAB], f16)
            nc.vector.memset(c_p1[96:128], 0.0)
            nc.vector.memset(c_m1[0:32], 0.0)
            nc.sync.dma_start(out=c_p1[0:127], in_=c_sb[1:128])
            nc.sync.dma_start(out=c_m1[1:128], in_=c_sb[0:127])
            c_c = ktp.tile([W, SLAB], f16)
            nc.vector.tensor_scalar(c_c, c_sb, ef_sb, None, Alu.mult)
            for dj, csrc in ((0, c_p1), (1, c_c), (2, c_m1)):
                cb = csrc.unsqueeze(1).broadcast_to((W, DIM, SLAB))
                prod = stp.tile([W, DIM, SLAB], f16, tag="prod")
                nc.vector.tensor_tensor(prod, s_sb, cb, Alu.mult)
                nc.vector.tensor_tensor(accs[dj], accs[dj], prod, Alu.add)

            # merge: out[w] = acc1[w] + acc0[w-1] + acc2[w+1] via partition
            # shifts (DMA), then one XBAR transpose of [w, (c r)]
            a0s = accp.tile([W, DIM, SLAB], f16)
            a2s = accp.tile([W, DIM, SLAB], f16)
            nc.vector.memset(a0s[0:32], 0.0)
            nc.vector.memset(a2s[96:128], 0.0)
            nc.sync.dma_start(out=a0s[1:128], in_=accs[0][0:127])
            nc.sync.dma_start(out=a2s[0:127], in_=accs[2][1:128])
            nc.vector.tensor_tensor(accs[1], accs[1], gacc, Alu.add)
            macc = accp.tile([W, DIM, SLAB], f16)
            nc.vector.tensor_tensor(macc, accs[1], a0s, Alu.add)
            nc.vector.tensor_tensor(macc, macc, a2s, Alu.add)

            obig = accp.tile([128, 16, 128], f16)
            nc.sync.dma_start(
                out=obig,
                in_=macc.rearrange("p (o a) r -> p o (a r)", o=16, a=4),
                transpose=True)
            # obig[m, o, w] = macc[w, 128o + m]; c = 4o + m//32, r = m%32
            out_v = out_d.rearrange("(o c4) r w -> (c4 r) o w", o=16, c4=4)
            nc.sync.dma_start(out=out_v, in_=obig)

    return nc


def _get_program():
    if "nc" not in _PROGRAM_CACHE:
        nc = _build_program()
        nc.finalize()
        _PROGRAM_CACHE["nc"] = nc
    return _PROGRAM_CACHE["nc"]


def _host_prep(x, w1, b1, w2, b2):
    """Build the 8 per-core input maps from full inputs."""
    x = np.asarray(x, dtype=np.float32)
    w1 = np.asarray(w1, dtype=np.float32)
    b1 = np.asarray(b1, dtype=np.float32)
    w2 = np.asarray(w2, dtype=np.float32)
    b2 = np.asarray(b2, dtype=np.float32)

    # stacked conv weights: row pairs di=a in partition halves; col pair
    # (di=2, dj=a); leftover (di=2, dj=2) in partitions 64..127
    w1s = np.zeros((128, 3, DIM), dtype=np.float16)
    w1r = np.zeros((128, 3, DIM), dtype=np.float16)
    w1q = np.zeros((128, DIM), dtype=np.float16)
    for a in range(2):
        w1s[64 * a:64 * a + 64] = w1[:, :, a, :].transpose(1, 2, 0)
        w1q[64 * a:64 * a + 64] = w1[:, :, 2, a].transpose(1, 0)
    w1r[64:128] = w1[:, :, 2, :].transpose(1, 2, 0)
    w2s = np.zeros((128, 3, 9), dtype=np.float16)
    w2r = np.zeros((128, 3, 9), dtype=np.float16)
    w2q = np.zeros((128, 9), dtype=np.float16)
    for a in range(2):
        w2s[64 * a:64 * a + 64] = w2[:, :, a, :].transpose(1, 2, 0)
        w2q[64 * a:64 * a + 64] = w2[:, :, 2, a].transpose(1, 0)
    w2r[64:128] = w2[:, :, 2, :].transpose(1, 2, 0)

    b1c = np.ascontiguousarray(b1.reshape(DIM, 1))
    b2c = np.ascontiguousarray(b2.reshape(9, 1))
    emask = np.zeros((W, 1), dtype=np.float32)
    emask[127, 0] = 1.0
    efold = np.ones((W, 1), dtype=np.float32)
    efold[0, 0] = 2.0
    efold[127, 0] = 2.0
    emA = np.zeros((W, 1), dtype=np.float32)
    emA[0, 0] = 1.0 / 45.0
    emB = np.zeros((W, 1), dtype=np.float32)
    emB[127, 0] = 1.0 / 45.0

    x16 = x.astype(np.float16)
    in_maps = []
    for core in range(NCORES):
        b, s = divmod(core, 4)
        r0 = s * SLAB
        # conv input, stacked: xs[c+64a, f, g, w] = xpad[c, f, r0-2+g+a, w-1]
        xs = np.zeros((128, T, GH, GW), dtype=np.float16)
        for a in range(2):
            lo = r0 - 2 + a
            hi = lo + GH            # rows lo .. hi-1
            clo = max(0, lo)
            chi = min(H, hi)
            if chi > clo:
                xs[64 * a:64 * a + 64, :, clo - lo:chi - lo, 1:129] = \
                    x16[b, :, :, clo:chi, :]
        # filter input, pixel-partitioned: xt[w, f, c, r]
        rows = np.clip(np.arange(r0 - 1, r0 + 33), 0, H - 1)
        xt = np.ascontiguousarray(
            x16[b][:, :, rows, :].transpose(3, 1, 0, 2))
        ymask = np.ones((DIM, 2), dtype=np.float32)
        if s == 0:
            ymask[:, 0] = 0.0
        if s == 3:
            ymask[:, 1] = 0.0
        in_maps.append({
            "xs": xs, "xt": xt, "w1s": w1s, "w1r": w1r, "w1q": w1q,
            "w2s": w2s, "w2r": w2r, "w2q": w2q, "b1c": b1c, "b2c": b2c,
            "ymask": ymask, "emask": emask, "efold": efold, "emA": emA,
            "emB": emB,
        })
    return in_maps


def kernel(x, w1, b1, w2, b2):
    from concourse.bass_utils import run_bass_kernel_spmd

    nc = _get_program()
    in_maps = _host_prep(x, w1, b1, w2, b2)
    res = run_bass_kernel_spmd(nc, in_maps, list(range(NCORES)))
    out = np.zeros((2, DIM, H, W), dtype=np.float32)
    for core in range(NCORES):
        b, s = divmod(core, 4)
        out[b, :, s * SLAB:(s + 1) * SLAB, :] = \
            res.results[core]["out"].astype(np.float32)
    return out


# revision 10
# speedup vs baseline: 1.1078x; 1.1078x over previous
"""Trainium2 Bass kernel for nn_DynamicFiltering (v3).

Computation (per batch b):
  y  = LeakyReLU(conv2d(x_f, w1, b1), 0.2)      per frame f
  ker = conv2d(y, w2, b2)                        (9, h, w) per frame
  ker = ker - mean_k(ker) + 1/45                 (K = t*3*3 = 45 per pixel)
  out[c,h,w] = sum_{t,k1,k2} x_edge[c,t,h+k1-1,w+k2-1] * ker[t,k1,k2][h,w]

Sharding: 8 cores = 2 batches x 4 H-slabs of 32 rows.

Design notes:
  - fp16 on-chip throughout (gate is 2e-2, fp16 keeps ~15x margin)
  - convs with contraction packed to 128: partitions 64..127 hold a
    row-shifted (xs/ys) or row+col-shifted (xs01/ys01) copy, so the 9-tap
    conv runs as 5 matmuls per 4-row chunk (3 row-pairs + 1 col-pair + 1
    single); the shifted copies are built by SBUF->SBUF DMA
  - LeakyReLU via the hardware Prelu activation (alpha=0.2)
  - ker (9,r,w) -> (w,r,9) pivot via one XBAR dma_start_transpose per frame
  - filtering layout [pix, c, r] (r contiguous) for DVE 16-bit mode; the
    tap products come from DVE tensor_tensor, the accumulations run as
    gpsimd software-DGE DMA chains with accum_op=add (one chain per dj),
    keeping the DVE free for the multiplies
  - final merge: partition-shift the dj accumulators by DMA, add, then one
    XBAR transpose of [w, (c r)]; out is fp16 and the host converts
"""

import numpy as np

DIM = 64
T = 5
H = 128
W = 128
SLAB = 32          # output rows per core
NCORES = 8
GH = 36            # conv grid rows: slab + 2*2 halo
GW = 130           # conv grid cols: W + 2
FR = 34            # filter rows: slab + 2 halo

USE_QCONV = True
USE_DMA_ACCUM = False
GP_TAPS = ()
_PROGRAM_CACHE = {}


def _build_program():
    import concourse.bacc as bacc
    import concourse.mybir as mybir
    from concourse.tile import TileContext

    f32 = mybir.dt.float32
    f16 = mybir.dt.float16
    Act = mybir.ActivationFunctionType
    Alu = mybir.AluOpType

    nc = bacc.Bacc("TRN2", debug=False)

    xs_d = nc.dram_tensor("xs", [128, T, GH, GW], f16, kind="ExternalInput").ap()
    xt_d = nc.dram_tensor("xt", [W, T, DIM, FR], f16, kind="ExternalInput").ap()
    xq_d = nc.dram_tensor("xq", [128, T, GH - 2, W], f16, kind="ExternalInput").ap()
    w1s_d = nc.dram_tensor("w1s", [128, 3, DIM], f16, kind="ExternalInput").ap()
    w1r_d = nc.dram_tensor("w1r", [128, 3, DIM], f16, kind="ExternalInput").ap()
    w1q_d = nc.dram_tensor("w1q", [128, DIM], f16, kind="ExternalInput").ap()
    w2s_d = nc.dram_tensor("w2s", [128, 3, 9], f16, kind="ExternalInput").ap()
    w2r_d = nc.dram_tensor("w2r", [128, 3, 9], f16, kind="ExternalInput").ap()
    w2q_d = nc.dram_tensor("w2q", [128, 9], f16, kind="ExternalInput").ap()
    b1_d = nc.dram_tensor("b1c", [DIM, 1], f32, kind="ExternalInput").ap()
    b2_d = nc.dram_tensor("b2c", [9, 1], f32, kind="ExternalInput").ap()
    ym_d = nc.dram_tensor("ymask", [DIM, 2], f32, kind="ExternalInput").ap()
    em_d = nc.dram_tensor("emask", [W, 1], f32, kind="ExternalInput").ap()
    ef_d = nc.dram_tensor("efold", [W, 1], f32, kind="ExternalInput").ap()
    ea_d = nc.dram_tensor("emA", [W, 1], f32, kind="ExternalInput").ap()
    eb_d = nc.dram_tensor("emB", [W, 1], f32, kind="ExternalInput").ap()
    out_d = nc.dram_tensor("out", [DIM, SLAB, W], f16, kind="ExternalOutput").ap()

    with nc.allow_low_precision(reason="2e-2 gate; fp16 has ~15x margin"), \
            TileContext(nc) as tc:
        with (
            tc.tile_pool(name="consts", bufs=1) as cpool,
            tc.tile_pool(name="xsp", bufs=3) as xsp,
            tc.tile_pool(name="xqp", bufs=2) as xqp,
            tc.tile_pool(name="ysp", bufs=2) as ysp,
            tc.tile_pool(name="yqp", bufs=2) as yqp,
            tc.tile_pool(name="kerp", bufs=2) as kerp,
            tc.tile_pool(name="kt1p", bufs=2) as kt1p,
            tc.tile_pool(name="ktp", bufs=1) as ktp,
            tc.tile_pool(name="accp", bufs=1) as accp,
            tc.tile_pool(name="stage", bufs=4) as stp,
        ):
            # --- loads needed by conv1(0) go first on the SP queue
            xs_f_tiles = {}

            def load_xs(f):
                t = xsp.tile([128, GH, GW], f16, tag="xs")
                nc.sync.dma_start(out=t, in_=xs_d[:, f])
                xs_f_tiles[f] = t

            xq_tiles = {}

            def load_xq(f):
                t = xqp.tile([128, GH - 2, W], f16, tag="xq")
                nc.sync.dma_start(out=t, in_=xq_d[:, f])
                xq_tiles[f] = t

            load_xs(0)
            load_xq(0)
            w1s_sb = cpool.tile([128, 3, DIM], f16)
            nc.sync.dma_start(out=w1s_sb, in_=w1s_d)
            w1r_sb = cpool.tile([128, 3, DIM], f16)
            nc.sync.dma_start(out=w1r_sb, in_=w1r_d)
            w1q_sb = cpool.tile([128, DIM], f16)
            nc.sync.dma_start(out=w1q_sb, in_=w1q_d)
            b1_sb = cpool.tile([DIM, 1], f32)
            nc.sync.dma_start(out=b1_sb, in_=b1_d)
            load_xs(1)
            load_xq(1)
            w2s_sb = cpool.tile([128, 3, 9], f16)
            nc.sync.dma_start(out=w2s_sb, in_=w2s_d)
            w2r_sb = cpool.tile([128, 3, 9], f16)
            nc.sync.dma_start(out=w2r_sb, in_=w2r_d)
            w2q_sb = cpool.tile([128, 9], f16)
            nc.sync.dma_start(out=w2q_sb, in_=w2q_d)
            b2_sb = cpool.tile([9, 1], f32)
            nc.sync.dma_start(out=b2_sb, in_=b2_d)
            ym_sb = cpool.tile([DIM, 2], f32)
            nc.sync.dma_start(out=ym_sb, in_=ym_d)

            # filter input frames (consumed from post(0) on)
            xt_sb = cpool.tile([W, T, DIM, FR], f16)
            for f in range(T):
                nc.sync.dma_start(out=xt_sb[:, f], in_=xt_d[:, f])

            em_sb = cpool.tile([W, 1], f32)
            nc.sync.dma_start(out=em_sb, in_=em_d)
            ef_sb = cpool.tile([W, 1], f32)
            nc.sync.dma_start(out=ef_sb, in_=ef_d)
            ea_sb = cpool.tile([W, 1], f32)
            nc.sync.dma_start(out=ea_sb, in_=ea_d)
            eb_sb = cpool.tile([W, 1], f32)
            nc.sync.dma_start(out=eb_sb, in_=eb_d)

            # per-pixel kernels, pixel-partitioned: kt[p, 9f+3di+dj, r]
            kt = ktp.tile([W, 48, SLAB], f16)
            ktr = kt[:, 0:45, :].rearrange(
                "p (t di dj) r -> p t di dj r", t=T, di=3, dj=3)
            kt_p1 = ktp.tile([W, 48, SLAB], f16)   # kt_p1[q] = kt[q+1]
            kt_m1 = ktp.tile([W, 48, SLAB], f16)   # kt_m1[q] = kt[q-1]
            nc.gpsimd.memset(kt_p1[96:128], 0.0)
            nc.gpsimd.memset(kt_m1[0:32], 0.0)

            # dj-separated accumulators (fp16), filled by DMA-accum chains
            accs = []
            for dj in range(3):
                a = accp.tile([W, DIM, SLAB], f16, name=f"acc{dj}")
                nc.gpsimd.memset(a, 0.0)
                accs.append(a)
            u_sb = accp.tile([W, DIM, FR], f16)
            nc.gpsimd.memset(u_sb, 0.0)

            ys_tiles = {}
            ker_tiles = {}

            with (
                tc.tile_pool(name="ps1", bufs=4, space="PSUM") as ps1p,
                tc.tile_pool(name="ps2", bufs=4, space="PSUM") as ps2p,
            ):
                def conv1(f):
                    xs_f = xs_f_tiles[f]
                    xq = xq_tiles[f]
                    ys = ysp.tile([128, GH, GW], f16, tag="ys")
                    ys_tiles[f] = ys
                    nc.gpsimd.memset(ys[0:64, 1:35, 0:1], 0.0)
                    nc.gpsimd.memset(ys[0:64, 1:35, 129:130], 0.0)
                    for rc in range(9):
                        g0 = 1 + 4 * rc
                        nr = 4 if rc < 8 else 2
                        ps = ps1p.tile([DIM, 4, W], f32, tag="ps1")
                        for dj in range(3):
                            nc.tensor.matmul(
                                ps[:, :nr, :],
                                lhsT=w1s_sb[:, dj, :],
                                rhs=xs_f[:, g0 - 1:g0 - 1 + nr, dj:dj + W],
                                start=(dj == 0), stop=False)
                        if USE_QCONV:
                            nc.tensor.matmul(
                                ps[:, :nr, :], lhsT=w1q_sb,
                                rhs=xq[:, g0 - 1:g0 - 1 + nr, :],
                                start=False, stop=False)
                            nc.tensor.matmul(
                                ps[:, :nr, :], lhsT=w1r_sb[64:128, 2, :],
                                rhs=xs_f[64:128, g0:g0 + nr, 2:2 + W],
                                start=False, stop=True)
                        else:
                            for dj in range(3):
                                nc.tensor.matmul(
                                    ps[:, :nr, :],
                                    lhsT=w1r_sb[64:128, dj, :],
                                    rhs=xs_f[64:128, g0:g0 + nr, dj:dj + W],
                                    start=False, stop=(dj == 2))
                        nc.scalar.activation(
                            ys[0:64, g0:g0 + nr, 1:129], ps[:, :nr],
                            Act.Prelu, bias=b1_sb, scale=1.0, alpha=0.2)
                    # zero y halo rows outside the image (conv2 zero-pad)
                    nc.scalar.activation(ys[0:64, 1:2, 1:129],
                                         ys[0:64, 1:2, 1:129],
                                         Act.Copy, scale=ym_sb[:, 0:1])
                    nc.scalar.activation(ys[0:64, 34:35, 1:129],
                                         ys[0:64, 34:35, 1:129],
                                         Act.Copy, scale=ym_sb[:, 1:2])
                    # stacked row-shifted copy: ys[64+c, g] = ys[c, g+1]
                    nc.sync.dma_start(out=ys[64:128, 1:34, :],
                                      in_=ys[0:64, 2:35, :])

                def conv2(f):
                    ys = ys_tiles[f]
                    # col-pair-stacked copy for taps (di=2, dj=0/1):
                    # yq[o+64a, g, w] = y[o, r0-2+g+2, w-1+a]
                    yq = yqp.tile([128, 32, W], f16, tag="yq")
                    nc.sync.dma_start(out=yq[0:64], in_=ys[0:64, 3:35, 0:128])
                    nc.sync.dma_start(out=yq[64:128],
                                      in_=ys[0:64, 3:35, 1:129])
                    ker_f = kerp.tile([16, SLAB, W], f16, tag="ker")
                    ker_tiles[f] = ker_f
                    for rc in range(8):
                        g0 = 2 + 4 * rc
                        ps2 = ps2p.tile([9, 4, W], f32, tag="ps2")
                        for dj in range(3):
                            nc.tensor.matmul(
                                ps2,
                                lhsT=w2s_sb[:, dj, :],
                                rhs=ys[:, g0 - 1:g0 + 3, dj:dj + W],
                                start=(dj == 0), stop=False)
                        if USE_QCONV:
                            nc.tensor.matmul(
                                ps2, lhsT=w2q_sb,
                                rhs=yq[:, g0 - 2:g0 + 2, :],
                                start=False, stop=False)
                            nc.tensor.matmul(
                                ps2, lhsT=w2r_sb[64:128, 2, :],
                                rhs=ys[64:128, g0:g0 + 4, 2:2 + W],
                                start=False, stop=True)
                        else:
                            for dj in range(3):
                                nc.tensor.matmul(
                                    ps2, lhsT=w2r_sb[64:128, dj, :],
                                    rhs=ys[64:128, g0:g0 + 4, dj:dj + W],
                                    start=False, stop=(dj == 2))
                        nc.scalar.activation(
                            ker_f[0:9, 4 * rc:4 * rc + 4, :], ps2,
                            Act.Identity, bias=b2_sb, scale=1.0)

                def post(f):
                    # pivot ker (9, r, w) -> (w, r, 9) via the XBAR
                    ker_f = ker_tiles[f]
                    kt1 = kt1p.tile([W, SLAB, 16], f16, tag="kt1")
                    nc.sync.dma_start(out=kt1, in_=ker_f, transpose=True)
                    # repack to (w, 9, r): taps outer, rows contiguous
                    nc.scalar.copy(
                        kt[:, 9 * f:9 * f + 9, :],
                        kt1[:, :, 0:9].rearrange("p r k -> p k r"))
                    # fold W-edge replicate-pad terms into the dj=1 slot
                    nc.vector.tensor_tensor(ktr[0:1, f, :, 1, :],
                                            ktr[0:1, f, :, 1, :],
                                            ktr[0:1, f, :, 0, :], Alu.add)
                    nc.vector.scalar_tensor_tensor(
                        out=ktr[96:128, f, :, 1, :],
                        in0=ktr[96:128, f, :, 2, :], scalar=em_sb[96:128, :],
                        in1=ktr[96:128, f, :, 1, :],
                        op0=Alu.mult, op1=Alu.add)
                    # partition-shifted kernel copies for dj=0 / dj=2 taps
                    nc.sync.dma_start(out=kt_p1[0:127, 9 * f:9 * f + 9, :],
                                      in_=kt[1:128, 9 * f:9 * f + 9, :])
                    nc.sync.dma_start(out=kt_m1[1:128, 9 * f:9 * f + 9, :],
                                      in_=kt[0:127, 9 * f:9 * f + 9, :])
                    # dynamic filtering: DVE multiplies, DMA-accum adds
                    ksrc = [kt_p1, kt, kt_m1]
                    for di in range(3):
                        for dj in range(3):
                            kb = ksrc[dj][:, 9 * f + 3 * di + dj, :]\
                                .unsqueeze(1).broadcast_to((W, DIM, SLAB))
                            xt_sl = xt_sb[:, f, :, di:di + SLAB]
                            prod = stp.tile([W, DIM, SLAB], f16,
                                            tag="prod")
                            nc.vector.tensor_tensor(prod, xt_sl, kb,
                                                    Alu.mult)
                            nc.vector.tensor_tensor(accs[dj], accs[dj],
                                                    prod, Alu.add)
                    # u += xt_f (for the normalization term c * S)
                    nc.vector.tensor_tensor(u_sb, u_sb, xt_sb[:, f],
                                            Alu.add)

                conv1(0)
                conv2(0)
                load_xs(2)
                load_xq(2)
                post(0)
                conv1(1)
                conv2(1)
                load_xs(3)
                load_xq(3)
                post(1)
                conv1(2)
                conv2(2)
                load_xs(4)
                load_xq(4)
                post(2)
                conv1(3)
                conv2(3)
                post(3)
                conv1(4)
                conv2(4)
                post(4)

            # normalization: out += c * S with c = 1/45 - mean(ker);
            # sum45 reads the folded kernel, undo the edge double-count
            sum45 = ktp.tile([W, SLAB], f16)
            kt_v = kt[:, 0:45, :].rearrange("p (t n) r -> p r t n", t=T)
            nc.vector.tensor_reduce(sum45, kt_v, axis=mybir.AxisListType.XY,
                                    op=Alu.add)
            c_sb = ktp.tile([W, SLAB], f16)
            nc.vector.tensor_scalar(c_sb, sum45, -1.0 / 45.0, 1.0 / 45.0,
                                    Alu.mult, Alu.add)
            corr = ktp.tile([W, SLAB], f16)
            kt_e = kt[:, 0:45, :].rearrange(
                "p (t di dj) r -> p r t di dj", t=T, di=3, dj=3)
            nc.vector.tensor_reduce(corr[0:32], kt_e[0:32, :, :, :, 0],
                                    axis=mybir.AxisListType.XY, op=Alu.add)
            nc.vector.tensor_reduce(corr[96:128], kt_e[96:128, :, :, :, 2],
                                    axis=mybir.AxisListType.XY, op=Alu.add)
            nc.vector.scalar_tensor_tensor(out=c_sb[0:32], in0=corr[0:32],
                                           scalar=ea_sb[0:32], in1=c_sb[0:32],
                                           op0=Alu.mult, op1=Alu.add)
            nc.vector.scalar_tensor_tensor(out=c_sb[96:128], in0=corr[96:128],
                                           scalar=eb_sb[96:128],
                                           in1=c_sb[96:128],
                                           op0=Alu.mult, op1=Alu.add)

            # S = 3-row vertical box of u (edge rows already clamped in xt)
            s_sb = accp.tile([W, DIM, SLAB], f16)
            nc.vector.tensor_tensor(s_sb, u_sb[:, :, 0:SLAB],
                                    u_sb[:, :, 1:SLAB + 1], Alu.add)
            nc.vector.tensor_tensor(s_sb, s_sb, u_sb[:, :, 2:SLAB + 2],
                                    Alu.add)

            # shifted + edge-doubled variants of c
            c_p1 = ktp.tile([W, SLAB], f16)
            c_m1 = ktp.tile([W, SLAB], f16)
            nc.gpsimd.memset(c_p1[96:128], 0.0)
            nc.gpsimd.memset(c_m1[0:32], 0.0)
            nc.sync.dma_start(out=c_p1[0:127], in_=c_sb[1:128])
            nc.sync.dma_start(out=c_m1[1:128], in_=c_sb[0:127])
            c_c = ktp.tile([W, SLAB], f16)
            nc.vector.tensor_scalar(c_c, c_sb, ef_sb, None, Alu.mult)
            for dj, csrc in ((0, c_p1), (1, c_c), (2, c_m1)):
                cb = csrc.unsqueeze(1).broadcast_to((W, DIM, SLAB))
                prod = stp.tile([W, DIM, SLAB], f16, tag="prod")
                nc.vector.tensor_tensor(prod, s_sb, cb, Alu.mult)
                nc.vector.tensor_tensor(accs[dj], accs[dj], prod, Alu.add)

            # merge: out[w] = acc1[w] + acc0[w-1] + acc2[w+1] via partition
            # shifts (DMA), then one XBAR transpose of [w, (c r)]
            a0s = accp.tile([W, DIM, SLAB], f16)
            a2s = accp.tile([W, DIM, SLAB], f16)
            nc.gpsimd.memset(a0s[0:32], 0.0)
            nc.gpsimd.memset(a2s[96:128], 0.0)
            nc.sync.dma_start(out=a0s[1:128], in_=accs[0][0:127])
            nc.sync.dma_start(out=a2s[0:127], in_=accs[2][1:128])
            macc = accp.tile([W, DIM, SLAB], f16)
            nc.vector.tensor_tensor(macc, accs[1], a0s, Alu.add)
            nc.vector.tensor_tensor(macc, macc, a2s, Alu.add)

            obig = accp.tile([128, 16, 128], f16)
            nc.sync.dma_start(
                out=obig,
                in_=macc.rearrange("p (o a) r -> p o (a r)", o=16, a=4),
                transpose=True)
            # obig[m, o, w] = macc[w, 128o + m]; c = 4o + m//32, r = m%32
            out_v = out_d.rearrange("(o c4) r w -> (c4 r) o w", o=16, c4=4)
            nc.sync.dma_start(out=out_v, in_=obig)

    return nc


def _get_program():
    if "nc" not in _PROGRAM_CACHE:
        nc = _build_program()
        nc.finalize()
        _PROGRAM_CACHE["nc"] = nc
    return _PROGRAM_CACHE["nc"]


def _host_prep(x, w1, b1, w2, b2):
    """Build the 8 per-core input maps from full inputs."""
    x = np.asarray(x, dtype=np.float32)
    w1 = np.asarray(w1, dtype=np.float32)
    b1 = np.asarray(b1, dtype=np.float32)
    w2 = np.asarray(w2, dtype=np.float32)
    b2 = np.asarray(b2, dtype=np.float32)

    # stacked conv weights: row pairs di=a in partition halves; col pair
    # (di=2, dj=a); leftover (di=2, dj=2) in partitions 64..127
    w1s = np.zeros((128, 3, DIM), dtype=np.float16)
    w1r = np.zeros((128, 3, DIM), dtype=np.float16)
    w1q = np.zeros((128, DIM), dtype=np.float16)
    for a in range(2):
        w1s[64 * a:64 * a + 64] = w1[:, :, a, :].transpose(1, 2, 0)
        w1q[64 * a:64 * a + 64] = w1[:, :, 2, a].transpose(1, 0)
    w1r[64:128] = w1[:, :, 2, :].transpose(1, 2, 0)
    w2s = np.zeros((128, 3, 9), dtype=np.float16)
    w2r = np.zeros((128, 3, 9), dtype=np.float16)
    w2q = np.zeros((128, 9), dtype=np.float16)
    for a in range(2):
        w2s[64 * a:64 * a + 64] = w2[:, :, a, :].transpose(1, 2, 0)
        w2q[64 * a:64 * a + 64] = w2[:, :, 2, a].transpose(1, 0)
    w2r[64:128] = w2[:, :, 2, :].transpose(1, 2, 0)

    b1c = np.ascontiguousarray(b1.reshape(DIM, 1))
    b2c = np.ascontiguousarray(b2.reshape(9, 1))
    emask = np.zeros((W, 1), dtype=np.float32)
    emask[127, 0] = 1.0
    efold = np.ones((W, 1), dtype=np.float32)
    efold[0, 0] = 2.0
    efold[127, 0] = 2.0
    emA = np.zeros((W, 1), dtype=np.float32)
    emA[0, 0] = 1.0 / 45.0
    emB = np.zeros((W, 1), dtype=np.float32)
    emB[127, 0] = 1.0 / 45.0

    x16 = x.astype(np.float16)
    in_maps = []
    for core in range(NCORES):
        b, s = divmod(core, 4)
        r0 = s * SLAB
        # conv input, stacked: xs[c+64a, f, g, w] = xpad[c, f, r0-2+g+a, w-1]
        xs = np.zeros((128, T, GH, GW), dtype=np.float16)
        for a in range(2):
            lo = r0 - 2 + a
            hi = lo + GH            # rows lo .. hi-1
            clo = max(0, lo)
            chi = min(H, hi)
            if chi > clo:
                xs[64 * a:64 * a + 64, :, clo - lo:chi - lo, 1:129] = \
                    x16[b, :, :, clo:chi, :]
        # filter input, pixel-partitioned: xt[w, f, c, r]
        rows = np.clip(np.arange(r0 - 1, r0 + 33), 0, H - 1)
        xt = np.ascontiguousarray(
            x16[b][:, :, rows, :].transpose(3, 1, 0, 2))
        # col-pair-stacked conv1 input: xq[c+64a, f, g, w] =
        #   xpad[c, f, r0+g, w-1+a]   (g = 0..33)
        xq = np.zeros((128, T, GH - 2, W), dtype=np.float16)
        for a in range(2):
            lo = r0
            hi = lo + GH - 2
            clo = max(0, lo)
            chi = min(H, hi)
            if chi > clo:
                if a == 0:
                    xq[0:64, :, clo - lo:chi - lo, 1:128] = \
                        x16[b, :, :, clo:chi, 0:127]
                else:
                    xq[64:128, :, clo - lo:chi - lo, 0:128] = \
                        x16[b, :, :, clo:chi, 0:128]
        ymask = np.ones((DIM, 2), dtype=np.float32)
        if s == 0:
            ymask[:, 0] = 0.0
        if s == 3:
            ymask[:, 1] = 0.0
        in_maps.append({
            "xs": xs, "xt": xt, "xq": xq, "w1s": w1s, "w1r": w1r, "w1q": w1q,
            "w2s": w2s, "w2r": w2r, "w2q": w2q, "b1c": b1c, "b2c": b2c,
            "ymask": ymask, "emask": emask, "efold": efold, "emA": emA,
            "emB": emB,
        })
    return in_maps


def kernel(x, w1, b1, w2, b2):
    from concourse.bass_utils import run_bass_kernel_spmd

    nc = _get_program()
    in_maps = _host_prep(x, w1, b1, w2, b2)
    res = run_bass_kernel_spmd(nc, in_maps, list(range(NCORES)))
    out = np.zeros((2, DIM, H, W), dtype=np.float32)
    for core in range(NCORES):
        b, s = divmod(core, 4)
        out[b, :, s * SLAB:(s + 1) * SLAB, :] = \
            res.results[core]["out"].astype(np.float32)
    return out


# revision 12
# speedup vs baseline: 1.2376x; 1.1171x over previous
"""Trainium2 Bass kernel for nn_DynamicFiltering (v3).

Computation (per batch b):
  y  = LeakyReLU(conv2d(x_f, w1, b1), 0.2)      per frame f
  ker = conv2d(y, w2, b2)                        (9, h, w) per frame
  ker = ker - mean_k(ker) + 1/45                 (K = t*3*3 = 45 per pixel)
  out[c,h,w] = sum_{t,k1,k2} x_edge[c,t,h+k1-1,w+k2-1] * ker[t,k1,k2][h,w]

Sharding: 8 cores = 2 batches x 4 H-slabs of 32 rows.

Design notes:
  - fp16 on-chip throughout (gate is 2e-2, fp16 keeps ~15x margin)
  - convs with contraction packed to 128: partitions 64..127 hold a
    row-shifted (xs/ys) or row+col-shifted (xs01/ys01) copy, so the 9-tap
    conv runs as 5 matmuls per 4-row chunk (3 row-pairs + 1 col-pair + 1
    single); the shifted copies are built by SBUF->SBUF DMA
  - LeakyReLU via the hardware Prelu activation (alpha=0.2)
  - ker (9,r,w) -> (w,r,9) pivot via one XBAR dma_start_transpose per frame
  - filtering layout [pix, c, r] (r contiguous) for DVE 16-bit mode; the
    tap products come from DVE tensor_tensor, the accumulations run as
    gpsimd software-DGE DMA chains with accum_op=add (one chain per dj),
    keeping the DVE free for the multiplies
  - final merge: partition-shift the dj accumulators by DMA, add, then one
    XBAR transpose of [w, (c r)]; out is fp16 and the host converts
"""

import numpy as np

DIM = 64
T = 5
H = 128
W = 128
SLAB = 32          # output rows per core
NCORES = 8
GH = 36            # conv grid rows: slab + 2*2 halo
GW = 130           # conv grid cols: W + 2
FR = 34            # filter rows: slab + 2 halo

USE_QCONV = True
USE_DMA_ACCUM = False
GP_TAPS = ()
_PROGRAM_CACHE = {}


def _build_program():
    import concourse.bacc as bacc
    import concourse.mybir as mybir
    from concourse.tile import TileContext

    f32 = mybir.dt.float32
    f16 = mybir.dt.float16
    Act = mybir.ActivationFunctionType
    Alu = mybir.AluOpType

    nc = bacc.Bacc("TRN2", debug=False)

    xs_d = nc.dram_tensor("xs", [128, T, GH, GW], f16, kind="ExternalInput").ap()
    xt_d = nc.dram_tensor("xt", [W, T, DIM, FR], f16, kind="ExternalInput").ap()
    xq_d = nc.dram_tensor("xq", [128, T, GH - 2, W], f16, kind="ExternalInput").ap()
    w1s_d = nc.dram_tensor("w1s", [128, 3, DIM], f16, kind="ExternalInput").ap()
    w1r_d = nc.dram_tensor("w1r", [128, 3, DIM], f16, kind="ExternalInput").ap()
    w1q_d = nc.dram_tensor("w1q", [128, DIM], f16, kind="ExternalInput").ap()
    w2s_d = nc.dram_tensor("w2s", [128, 3, 9], f16, kind="ExternalInput").ap()
    w2r_d = nc.dram_tensor("w2r", [128, 3, 9], f16, kind="ExternalInput").ap()
    w2q_d = nc.dram_tensor("w2q", [128, 9], f16, kind="ExternalInput").ap()
    b1_d = nc.dram_tensor("b1c", [DIM, 1], f32, kind="ExternalInput").ap()
    b2_d = nc.dram_tensor("b2c", [9, 1], f32, kind="ExternalInput").ap()
    ym_d = nc.dram_tensor("ymask", [DIM, 2], f32, kind="ExternalInput").ap()
    em_d = nc.dram_tensor("emask", [W, 1], f32, kind="ExternalInput").ap()
    ef_d = nc.dram_tensor("efold", [W, 1], f32, kind="ExternalInput").ap()
    ea_d = nc.dram_tensor("emA", [W, 1], f32, kind="ExternalInput").ap()
    eb_d = nc.dram_tensor("emB", [W, 1], f32, kind="ExternalInput").ap()
    out_d = nc.dram_tensor("out", [DIM, SLAB, W], f16, kind="ExternalOutput").ap()

    with nc.allow_low_precision(reason="2e-2 gate; fp16 has ~15x margin"), \
            TileContext(nc) as tc:
        with (
            tc.tile_pool(name="consts", bufs=1) as cpool,
            tc.tile_pool(name="xsp", bufs=3) as xsp,
            tc.tile_pool(name="xqp", bufs=2) as xqp,
            tc.tile_pool(name="ysp", bufs=2) as ysp,
            tc.tile_pool(name="yqp", bufs=2) as yqp,
            tc.tile_pool(name="kerp", bufs=2) as kerp,
            tc.tile_pool(name="kt1p", bufs=2) as kt1p,
            tc.tile_pool(name="ktp", bufs=1) as ktp,
            tc.tile_pool(name="accp", bufs=1) as accp,
            tc.tile_pool(name="stage", bufs=4) as stp,
        ):
            # --- loads needed by conv1(0) go first on the SP queue
            xs_f_tiles = {}

            def load_xs(f):
                t = xsp.tile([128, GH, GW], f16, tag="xs")
                nc.sync.dma_start(out=t, in_=xs_d[:, f])
                xs_f_tiles[f] = t

            xq_tiles = {}

            def load_xq(f):
                t = xqp.tile([128, GH - 2, W], f16, tag="xq")
                nc.sync.dma_start(out=t, in_=xq_d[:, f])
                xq_tiles[f] = t

            load_xs(0)
            load_xq(0)
            w1s_sb = cpool.tile([128, 3, DIM], f16)
            nc.sync.dma_start(out=w1s_sb, in_=w1s_d)
            w1r_sb = cpool.tile([128, 3, DIM], f16)
            nc.sync.dma_start(out=w1r_sb, in_=w1r_d)
            w1q_sb = cpool.tile([128, DIM], f16)
            nc.sync.dma_start(out=w1q_sb, in_=w1q_d)
            b1_sb = cpool.tile([DIM, 1], f32)
            nc.sync.dma_start(out=b1_sb, in_=b1_d)
            load_xs(1)
            load_xq(1)
            w2s_sb = cpool.tile([128, 3, 9], f16)
            nc.sync.dma_start(out=w2s_sb, in_=w2s_d)
            w2r_sb = cpool.tile([128, 3, 9], f16)
            nc.sync.dma_start(out=w2r_sb, in_=w2r_d)
            w2q_sb = cpool.tile([128, 9], f16)
            nc.sync.dma_start(out=w2q_sb, in_=w2q_d)
            b2_sb = cpool.tile([9, 1], f32)
            nc.sync.dma_start(out=b2_sb, in_=b2_d)
            ym_sb = cpool.tile([DIM, 2], f32)
            nc.sync.dma_start(out=ym_sb, in_=ym_d)

            # filter input frames (consumed from post(0) on)
            xt_sb = cpool.tile([W, T, DIM, FR], f16)
            for f in range(T):
                nc.sync.dma_start(out=xt_sb[:, f], in_=xt_d[:, f])

            em_sb = cpool.tile([W, 1], f32)
            nc.sync.dma_start(out=em_sb, in_=em_d)
            ef_sb = cpool.tile([W, 1], f32)
            nc.sync.dma_start(out=ef_sb, in_=ef_d)
            ea_sb = cpool.tile([W, 1], f32)
            nc.sync.dma_start(out=ea_sb, in_=ea_d)
            eb_sb = cpool.tile([W, 1], f32)
            nc.sync.dma_start(out=eb_sb, in_=eb_d)

            # per-pixel kernels, pixel-partitioned: kt[p, 9f+3di+dj, r]
            kt = ktp.tile([W, 48, SLAB], f16)
            ktr = kt[:, 0:45, :].rearrange(
                "p (t di dj) r -> p t di dj r", t=T, di=3, dj=3)
            kt_p1 = ktp.tile([W, 48, SLAB], f16)   # kt_p1[q] = kt[q+1]
            kt_m1 = ktp.tile([W, 48, SLAB], f16)   # kt_m1[q] = kt[q-1]
            nc.gpsimd.memset(kt_p1[96:128], 0.0)
            nc.gpsimd.memset(kt_m1[0:32], 0.0)

            # dj-separated accumulators (fp16), filled by DMA-accum chains
            accs = []
            for dj in range(3):
                a = accp.tile([W, DIM, SLAB], f16, name=f"acc{dj}")
                nc.gpsimd.memset(a, 0.0)
                accs.append(a)
            u_sb = accp.tile([W, DIM, FR], f16)
            nc.gpsimd.memset(u_sb, 0.0)

            ys_tiles = {}
            ker_tiles = {}

            with (
                tc.tile_pool(name="ps1", bufs=4, space="PSUM") as ps1p,
                tc.tile_pool(name="ps2", bufs=4, space="PSUM") as ps2p,
            ):
                def conv1(f):
                    xs_f = xs_f_tiles[f]
                    xq = xq_tiles[f]
                    ys = ysp.tile([128, GH, GW], f16, tag="ys")
                    ys_tiles[f] = ys
                    nc.gpsimd.memset(ys[0:64, 1:35, 0:1], 0.0)
                    nc.gpsimd.memset(ys[0:64, 1:35, 129:130], 0.0)
                    for rc in range(9):
                        g0 = 1 + 4 * rc
                        nr = 4 if rc < 8 else 2
                        ps = ps1p.tile([DIM, 4, W], f32, tag="ps1")
                        for dj in range(3):
                            nc.tensor.matmul(
                                ps[:, :nr, :],
                                lhsT=w1s_sb[:, dj, :],
                                rhs=xs_f[:, g0 - 1:g0 - 1 + nr, dj:dj + W],
                                start=(dj == 0), stop=False)
                        if USE_QCONV:
                            nc.tensor.matmul(
                                ps[:, :nr, :], lhsT=w1q_sb,
                                rhs=xq[:, g0 - 1:g0 - 1 + nr, :],
                                start=False, stop=False)
                            nc.tensor.matmul(
                                ps[:, :nr, :], lhsT=w1r_sb[64:128, 2, :],
                                rhs=xs_f[64:128, g0:g0 + nr, 2:2 + W],
                                start=False, stop=True)
                        else:
                            for dj in range(3):
                                nc.tensor.matmul(
                                    ps[:, :nr, :],
                                    lhsT=w1r_sb[64:128, dj, :],
                                    rhs=xs_f[64:128, g0:g0 + nr, dj:dj + W],
                                    start=False, stop=(dj == 2))
                        nc.scalar.activation(
                            ys[0:64, g0:g0 + nr, 1:129], ps[:, :nr],
                            Act.Prelu, bias=b1_sb, scale=1.0, alpha=0.2)
                    # zero y halo rows outside the image (conv2 zero-pad)
                    nc.scalar.activation(ys[0:64, 1:2, 1:129],
                                         ys[0:64, 1:2, 1:129],
                                         Act.Copy, scale=ym_sb[:, 0:1])
                    nc.scalar.activation(ys[0:64, 34:35, 1:129],
                                         ys[0:64, 34:35, 1:129],
                                         Act.Copy, scale=ym_sb[:, 1:2])
                    # stacked row-shifted copy: ys[64+c, g] = ys[c, g+1]
                    nc.sync.dma_start(out=ys[64:128, 1:34, :],
                                      in_=ys[0:64, 2:35, :])

                def conv2(f):
                    ys = ys_tiles[f]
                    # col-pair-stacked copy for taps (di=2, dj=0/1):
                    # yq[o+64a, g, w] = y[o, r0-2+g+2, w-1+a]
                    yq = yqp.tile([128, 32, W], f16, tag="yq")
                    nc.sync.dma_start(out=yq[0:64], in_=ys[0:64, 3:35, 0:128])
                    nc.sync.dma_start(out=yq[64:128],
                                      in_=ys[0:64, 3:35, 1:129])
                    ker_f = kerp.tile([16, SLAB, W], f16, tag="ker")
                    ker_tiles[f] = ker_f
                    for rc in range(8):
                        g0 = 2 + 4 * rc
                        ps2 = ps2p.tile([9, 4, W], f32, tag="ps2")
                        for dj in range(3):
                            nc.tensor.matmul(
                                ps2,
                                lhsT=w2s_sb[:, dj, :],
                                rhs=ys[:, g0 - 1:g0 + 3, dj:dj + W],
                                start=(dj == 0), stop=False)
                        if USE_QCONV:
                            nc.tensor.matmul(
                                ps2, lhsT=w2q_sb,
                                rhs=yq[:, g0 - 2:g0 + 2, :],
                                start=False, stop=False)
                            nc.tensor.matmul(
                                ps2, lhsT=w2r_sb[64:128, 2, :],
                                rhs=ys[64:128, g0:g0 + 4, 2:2 + W],
                                start=False, stop=True)
                        else:
                            for dj in range(3):
                                nc.tensor.matmul(
                                    ps2, lhsT=w2r_sb[64:128, dj, :],
                                    rhs=ys[64:128, g0:g0 + 4, dj:dj + W],
                                    start=False, stop=(dj == 2))
                        nc.scalar.activation(
                            ker_f[0:9, 4 * rc:4 * rc + 4, :], ps2,
                            Act.Identity, bias=b2_sb, scale=1.0)

                def post(f):
                    # pivot ker (9, r, w) -> (w, r, 9) via the XBAR
                    ker_f = ker_tiles[f]
                    kt1 = kt1p.tile([W, SLAB, 16], f16, tag="kt1")
                    nc.sync.dma_start(out=kt1, in_=ker_f, transpose=True)
                    # repack to (w, 9, r): taps outer, rows contiguous
                    nc.scalar.copy(
                        kt[:, 9 * f:9 * f + 9, :],
                        kt1[:, :, 0:9].rearrange("p r k -> p k r"))
                    # fold W-edge replicate-pad terms into the dj=1 slot
                    nc.vector.tensor_tensor(ktr[0:1, f, :, 1, :],
                                            ktr[0:1, f, :, 1, :],
                                            ktr[0:1, f, :, 0, :], Alu.add)
                    nc.vector.scalar_tensor_tensor(
                        out=ktr[96:128, f, :, 1, :],
                        in0=ktr[96:128, f, :, 2, :], scalar=em_sb[96:128, :],
                        in1=ktr[96:128, f, :, 1, :],
                        op0=Alu.mult, op1=Alu.add)
                    # partition-shifted kernel copies for dj=0 / dj=2 taps
                    nc.sync.dma_start(out=kt_p1[0:127, 9 * f:9 * f + 9, :],
                                      in_=kt[1:128, 9 * f:9 * f + 9, :])
                    nc.sync.dma_start(out=kt_m1[1:128, 9 * f:9 * f + 9, :],
                                      in_=kt[0:127, 9 * f:9 * f + 9, :])
                    # dynamic filtering: DVE multiplies, DMA-accum adds
                    ksrc = [kt_p1, kt, kt_m1]
                    for di in range(3):
                        for dj in range(3):
                            kb = ksrc[dj][:, 9 * f + 3 * di + dj, :]\
                                .unsqueeze(1).broadcast_to((W, DIM, SLAB))
                            xt_sl = xt_sb[:, f, :, di:di + SLAB]
                            prod = stp.tile([W, DIM, SLAB], f16,
                                            tag="prod")
                            nc.vector.tensor_tensor(prod, xt_sl, kb,
                                                    Alu.mult)
                            nc.vector.tensor_tensor(accs[dj], accs[dj],
                                                    prod, Alu.add)
                    # u += xt_f (for the normalization term c * S)
                    nc.vector.tensor_tensor(u_sb, u_sb, xt_sb[:, f],
                                            Alu.add)

                conv1(0)
                conv2(0)
                load_xs(2)
                load_xq(2)
                post(0)
                conv1(1)
                conv2(1)
                load_xs(3)
                load_xq(3)
                post(1)
                conv1(2)
                conv2(2)
                load_xs(4)
                load_xq(4)
                post(2)
                conv1(3)
                conv2(3)
                post(3)
                conv1(4)
                conv2(4)
                post(4)

            # normalization: out += c * S with c = 1/45 - mean(ker);
            # sum45 reads the folded kernel, undo the edge double-count
            sum45 = ktp.tile([W, SLAB], f16)
            kt_v = kt[:, 0:45, :].rearrange("p (t n) r -> p r t n", t=T)
            nc.vector.tensor_reduce(sum45, kt_v, axis=mybir.AxisListType.XY,
                                    op=Alu.add)
            c_sb = ktp.tile([W, SLAB], f16)
            nc.vector.tensor_scalar(c_sb, sum45, -1.0 / 45.0, 1.0 / 45.0,
                                    Alu.mult, Alu.add)
            corr = ktp.tile([W, SLAB], f16)
            kt_e = kt[:, 0:45, :].rearrange(
                "p (t di dj) r -> p r t di dj", t=T, di=3, dj=3)
            nc.vector.tensor_reduce(corr[0:32], kt_e[0:32, :, :, :, 0],
                                    axis=mybir.AxisListType.XY, op=Alu.add)
            nc.vector.tensor_reduce(corr[96:128], kt_e[96:128, :, :, :, 2],
                                    axis=mybir.AxisListType.XY, op=Alu.add)
            nc.vector.scalar_tensor_tensor(out=c_sb[0:32], in0=corr[0:32],
                                           scalar=ea_sb[0:32], in1=c_sb[0:32],
                                           op0=Alu.mult, op1=Alu.add)
            nc.vector.scalar_tensor_tensor(out=c_sb[96:128], in0=corr[96:128],
                                           scalar=eb_sb[96:128],
                                           in1=c_sb[96:128],
                                           op0=Alu.mult, op1=Alu.add)

            # S = 3-row vertical box of u (edge rows already clamped in xt)
            s_sb = accp.tile([W, DIM, SLAB], f16)
            nc.vector.tensor_tensor(s_sb, u_sb[:, :, 0:SLAB],
                                    u_sb[:, :, 1:SLAB + 1], Alu.add)
            nc.vector.tensor_tensor(s_sb, s_sb, u_sb[:, :, 2:SLAB + 2],
                                    Alu.add)

            # shifted + edge-doubled variants of c
            c_p1 = ktp.tile([W, SLAB], f16)
            c_m1 = ktp.tile([W, SLAB], f16)
            nc.gpsimd.memset(c_p1[96:128], 0.0)
            nc.gpsimd.memset(c_m1[0:32], 0.0)
            nc.sync.dma_start(out=c_p1[0:127], in_=c_sb[1:128])
            nc.sync.dma_start(out=c_m1[1:128], in_=c_sb[0:127])
            c_c = ktp.tile([W, SLAB], f16)
            nc.vector.tensor_scalar(c_c, c_sb, ef_sb, None, Alu.mult)
            for dj, csrc in ((0, c_p1), (1, c_c), (2, c_m1)):
                cb = csrc.unsqueeze(1).broadcast_to((W, DIM, SLAB))
                prod = stp.tile([W, DIM, SLAB], f16, tag="prod")
                nc.vector.tensor_tensor(prod, s_sb, cb, Alu.mult)
                nc.vector.tensor_tensor(accs[dj], accs[dj], prod, Alu.add)

            # merge: out[w] = acc1[w] + acc0[w-1] + acc2[w+1] via partition
            # shifts (DMA), then one XBAR transpose of [w, (c r)]
            a0s = accp.tile([W, DIM, SLAB], f16)
            a2s = accp.tile([W, DIM, SLAB], f16)
            nc.gpsimd.memset(a0s[0:32], 0.0)
            nc.gpsimd.memset(a2s[96:128], 0.0)
            nc.sync.dma_start(out=a0s[1:128], in_=accs[0][0:127])
            nc.sync.dma_start(out=a2s[0:127], in_=accs[2][1:128])
            macc = accp.tile([W, DIM, SLAB], f16)
            nc.vector.tensor_tensor(macc, accs[1], a0s, Alu.add)
            nc.vector.tensor_tensor(macc, macc, a2s, Alu.add)

            obig = accp.tile([128, 16, 128], f16)
            nc.sync.dma_start(
                out=obig,
                in_=macc.rearrange("p (o a) r -> p o (a r)", o=16, a=4),
                transpose=True)
            # obig[m, o, w] = macc[w, 128o + m]; c = 4o + m//32, r = m%32
            out_v = out_d.rearrange("(o c4) r w -> (c4 r) o w", o=16, c4=4)
            nc.sync.dma_start(out=out_v, in_=obig)

    return nc


def _get_program():
    if "nc" not in _PROGRAM_CACHE:
        nc = _build_program()
        nc.finalize()
        _PROGRAM_CACHE["nc"] = nc
    return _PROGRAM_CACHE["nc"]


def _host_prep(x, w1, b1, w2, b2):
    """Build the 8 per-core input maps from full inputs."""
    x = np.asarray(x, dtype=np.float32)
    w1 = np.asarray(w1, dtype=np.float32)
    b1 = np.asarray(b1, dtype=np.float32)
    w2 = np.asarray(w2, dtype=np.float32)
    b2 = np.asarray(b2, dtype=np.float32)

    # stacked conv weights: row pairs di=a in partition halves; col pair
    # (di=2, dj=a); leftover (di=2, dj=2) in partitions 64..127
    w1s = np.zeros((128, 3, DIM), dtype=np.float16)
    w1r = np.zeros((128, 3, DIM), dtype=np.float16)
    w1q = np.zeros((128, DIM), dtype=np.float16)
    for a in range(2):
        w1s[64 * a:64 * a + 64] = w1[:, :, a, :].transpose(1, 2, 0)
        w1q[64 * a:64 * a + 64] = w1[:, :, 2, a].transpose(1, 0)
    w1r[64:128] = w1[:, :, 2, :].transpose(1, 2, 0)
    w2s = np.zeros((128, 3, 9), dtype=np.float16)
    w2r = np.zeros((128, 3, 9), dtype=np.float16)
    w2q = np.zeros((128, 9), dtype=np.float16)
    for a in range(2):
        w2s[64 * a:64 * a + 64] = w2[:, :, a, :].transpose(1, 2, 0)
        w2q[64 * a:64 * a + 64] = w2[:, :, 2, a].transpose(1, 0)
    w2r[64:128] = w2[:, :, 2, :].transpose(1, 2, 0)

    b1c = np.ascontiguousarray(b1.reshape(DIM, 1))
    b2c = np.ascontiguousarray(b2.reshape(9, 1))
    emask = np.zeros((W, 1), dtype=np.float32)
    emask[127, 0] = 1.0
    efold = np.ones((W, 1), dtype=np.float32)
    efold[0, 0] = 2.0
    efold[127, 0] = 2.0
    emA = np.zeros((W, 1), dtype=np.float32)
    emA[0, 0] = 1.0 / 45.0
    emB = np.zeros((W, 1), dtype=np.float32)
    emB[127, 0] = 1.0 / 45.0

    x16 = x.astype(np.float16)
    in_maps = []
    for core in range(NCORES):
        b, s = divmod(core, 4)
        r0 = s * SLAB
        # conv input, stacked: xs[c+64a, f, g, w] = xpad[c, f, r0-2+g+a, w-1]
        xs = np.zeros((128, T, GH, GW), dtype=np.float16)
        for a in range(2):
            lo = r0 - 2 + a
            hi = lo + GH            # rows lo .. hi-1
            clo = max(0, lo)
            chi = min(H, hi)
            if chi > clo:
                xs[64 * a:64 * a + 64, :, clo - lo:chi - lo, 1:129] = \
                    x16[b, :, :, clo:chi, :]
        # filter input, pixel-partitioned: xt[w, f, c, r]
        rows = np.clip(np.arange(r0 - 1, r0 + 33), 0, H - 1)
        xt = np.ascontiguousarray(
            x16[b][:, :, rows, :].transpose(3, 1, 0, 2))
        # col-pair-stacked conv1 input: xq[c+64a, f, g, w] =
        #   xpad[c, f, r0+g, w-1+a]   (g = 0..33)
        xq = np.zeros((128, T, GH - 2, W), dtype=np.float16)
        for a in range(2):
            lo = r0
            hi = lo + GH - 2
            clo = max(0, lo)
            chi = min(H, hi)
            if chi > clo:
                if a == 0:
                    xq[0:64, :, clo - lo:chi - lo, 1:128] = \
                        x16[b, :, :, clo:chi, 0:127]
                else:
                    xq[64:128, :, clo - lo:chi - lo, 0:128] = \
                        x16[b, :, :, clo:chi, 0:128]
        ymask = np.ones((DIM, 2), dtype=np.float32)
        if s == 0:
            ymask[:, 0] = 0.0
        if s == 3:
            ymask[:, 1] = 0.0
        in_maps.append({
            "xs": xs, "xt": xt, "xq": xq, "w1s": w1s, "w1r": w1r, "w1q": w1q,
            "w2s": w2s, "w2r": w2r, "w2q": w2q, "b1c": b1c, "b2c": b2c,
            "ymask": ymask, "emask": emask, "efold": efold, "emA": emA,
            "emB": emB,
        })
    return in_maps


def kernel(x, w1, b1, w2, b2):
    from concourse.bass_utils import run_bass_kernel_spmd

    nc = _get_program()
    in_maps = _host_prep(x, w1, b1, w2, b2)
    res = run_bass_kernel_spmd(nc, in_maps, list(range(NCORES)))
    out = np.zeros((2, DIM, H, W), dtype=np.float32)
    for core in range(NCORES):
        b, s = divmod(core, 4)
        out[b, :, s * SLAB:(s + 1) * SLAB, :] = \
            res.results[core]["out"].astype(np.float32)
    return out


# revision 14
# speedup vs baseline: 1.3389x; 1.0819x over previous
"""Trainium2 Bass kernel for nn_DynamicFiltering (v3).

Computation (per batch b):
  y  = LeakyReLU(conv2d(x_f, w1, b1), 0.2)      per frame f
  ker = conv2d(y, w2, b2)                        (9, h, w) per frame
  ker = ker - mean_k(ker) + 1/45                 (K = t*3*3 = 45 per pixel)
  out[c,h,w] = sum_{t,k1,k2} x_edge[c,t,h+k1-1,w+k2-1] * ker[t,k1,k2][h,w]

Sharding: 8 cores = 2 batches x 4 H-slabs of 32 rows.

Design notes:
  - fp16 on-chip throughout (gate is 2e-2, fp16 keeps ~15x margin)
  - convs with contraction packed to 128: partitions 64..127 hold a
    row-shifted (xs/ys) or row+col-shifted (xs01/ys01) copy, so the 9-tap
    conv runs as 5 matmuls per 4-row chunk (3 row-pairs + 1 col-pair + 1
    single); the shifted copies are built by SBUF->SBUF DMA
  - LeakyReLU via the hardware Prelu activation (alpha=0.2)
  - ker (9,r,w) -> (w,r,9) pivot via one XBAR dma_start_transpose per frame
  - filtering layout [pix, c, r] (r contiguous) for DVE 16-bit mode; the
    tap products come from DVE tensor_tensor, the accumulations run as
    gpsimd software-DGE DMA chains with accum_op=add (one chain per dj),
    keeping the DVE free for the multiplies
  - final merge: partition-shift the dj accumulators by DMA, add, then one
    XBAR transpose of [w, (c r)]; out is fp16 and the host converts
"""

import numpy as np

DIM = 64
T = 5
H = 128
W = 128
SLAB = 32          # output rows per core
NCORES = 8
GH = 36            # conv grid rows: slab + 2*2 halo
GW = 130           # conv grid cols: W + 2
FR = 34            # filter rows: slab + 2 halo

USE_QCONV = True
USE_DMA_ACCUM = False
GP_TAPS = ()
_PROGRAM_CACHE = {}


def _build_program():
    import concourse.bacc as bacc
    import concourse.mybir as mybir
    from concourse.tile import TileContext

    f32 = mybir.dt.float32
    f16 = mybir.dt.float16
    Act = mybir.ActivationFunctionType
    Alu = mybir.AluOpType

    nc = bacc.Bacc("TRN2", debug=False)

    xs_d = nc.dram_tensor("xs", [128, T, GH, GW], f16, kind="ExternalInput").ap()
    xt_d = nc.dram_tensor("xt", [W, T, DIM, FR], f16, kind="ExternalInput").ap()
    xq_d = nc.dram_tensor("xq", [128, T, GH - 2, W], f16, kind="ExternalInput").ap()
    w1s_d = nc.dram_tensor("w1s", [128, 3, DIM], f16, kind="ExternalInput").ap()
    w1r_d = nc.dram_tensor("w1r", [128, 3, DIM], f16, kind="ExternalInput").ap()
    w1q_d = nc.dram_tensor("w1q", [128, DIM], f16, kind="ExternalInput").ap()
    w2s_d = nc.dram_tensor("w2s", [128, 3, 9], f16, kind="ExternalInput").ap()
    w2r_d = nc.dram_tensor("w2r", [128, 3, 9], f16, kind="ExternalInput").ap()
    b1_d = nc.dram_tensor("b1c", [DIM, 1], f32, kind="ExternalInput").ap()
    b2_d = nc.dram_tensor("b2c", [9, 1], f32, kind="ExternalInput").ap()
    ym_d = nc.dram_tensor("ymask", [DIM, 2], f32, kind="ExternalInput").ap()
    em_d = nc.dram_tensor("emask", [W, 1], f32, kind="ExternalInput").ap()
    ef_d = nc.dram_tensor("efold", [W, 1], f32, kind="ExternalInput").ap()
    ea_d = nc.dram_tensor("emA", [W, 1], f32, kind="ExternalInput").ap()
    eb_d = nc.dram_tensor("emB", [W, 1], f32, kind="ExternalInput").ap()
    out_d = nc.dram_tensor("out", [DIM, SLAB, W], f16, kind="ExternalOutput").ap()

    with nc.allow_low_precision(reason="2e-2 gate; fp16 has ~15x margin"), \
            TileContext(nc) as tc:
        with (
            tc.tile_pool(name="consts", bufs=1) as cpool,
            tc.tile_pool(name="xsp", bufs=3) as xsp,
            tc.tile_pool(name="xqp", bufs=2) as xqp,
            tc.tile_pool(name="ysp", bufs=2) as ysp,
            tc.tile_pool(name="kerp", bufs=2) as kerp,
            tc.tile_pool(name="kt1p", bufs=2) as kt1p,
            tc.tile_pool(name="ktp", bufs=1) as ktp,
            tc.tile_pool(name="accp", bufs=1) as accp,
            tc.tile_pool(name="stage", bufs=4) as stp,
        ):
            # --- loads needed by conv1(0) go first on the SP queue
            xs_f_tiles = {}

            def load_xs(f):
                t = xsp.tile([128, GH, GW], f16, tag="xs")
                nc.sync.dma_start(out=t, in_=xs_d[:, f])
                xs_f_tiles[f] = t

            xq_tiles = {}

            def load_xq(f):
                t = xqp.tile([128, GH - 2, W], f16, tag="xq")
                nc.sync.dma_start(out=t, in_=xq_d[:, f])
                xq_tiles[f] = t

            load_xs(0)
            load_xq(0)
            w1s_sb = cpool.tile([128, 3, DIM], f16)
            nc.sync.dma_start(out=w1s_sb, in_=w1s_d)
            w1r_sb = cpool.tile([128, 3, DIM], f16)
            nc.sync.dma_start(out=w1r_sb, in_=w1r_d)
            w1q_sb = cpool.tile([128, DIM], f16)
            nc.sync.dma_start(out=w1q_sb, in_=w1q_d)
            b1_sb = cpool.tile([DIM, 1], f32)
            nc.sync.dma_start(out=b1_sb, in_=b1_d)
            load_xs(1)
            load_xq(1)
            w2s_sb = cpool.tile([128, 3, 9], f16)
            nc.sync.dma_start(out=w2s_sb, in_=w2s_d)
            w2r_sb = cpool.tile([128, 3, 9], f16)
            nc.sync.dma_start(out=w2r_sb, in_=w2r_d)
            b2_sb = cpool.tile([9, 1], f32)
            nc.sync.dma_start(out=b2_sb, in_=b2_d)
            ym_sb = cpool.tile([DIM, 2], f32)
            nc.sync.dma_start(out=ym_sb, in_=ym_d)

            # filter input frames (consumed from post(0) on)
            xt_sb = cpool.tile([W, T, DIM, FR], f16)
            for f in range(T):
                nc.sync.dma_start(out=xt_sb[:, f], in_=xt_d[:, f])

            em_sb = cpool.tile([W, 1], f32)
            nc.sync.dma_start(out=em_sb, in_=em_d)
            ef_sb = cpool.tile([W, 1], f32)
            nc.sync.dma_start(out=ef_sb, in_=ef_d)
            ea_sb = cpool.tile([W, 1], f32)
            nc.sync.dma_start(out=ea_sb, in_=ea_d)
            eb_sb = cpool.tile([W, 1], f32)
            nc.sync.dma_start(out=eb_sb, in_=eb_d)

            # per-pixel kernels, pixel-partitioned: kt[p, 9f+3di+dj, r]
            kt = ktp.tile([W, 48, SLAB], f16)
            ktr = kt[:, 0:45, :].rearrange(
                "p (t di dj) r -> p t di dj r", t=T, di=3, dj=3)
            kt_p1 = ktp.tile([W, 48, SLAB], f16)   # kt_p1[q] = kt[q+1]
            kt_m1 = ktp.tile([W, 48, SLAB], f16)   # kt_m1[q] = kt[q-1]

            # dj-separated accumulators (fp16), filled by DMA-accum chains
            accs = []
            for dj in range(3):
                a = accp.tile([W, DIM, SLAB], f16, name=f"acc{dj}")
                nc.gpsimd.memset(a, 0.0)
                accs.append(a)
            u_sb = accp.tile([W, DIM, FR], f16)
            nc.gpsimd.memset(u_sb, 0.0)

            ys_tiles = {}
            ker_tiles = {}

            with (
                tc.tile_pool(name="ps1", bufs=4, space="PSUM") as ps1p,
                tc.tile_pool(name="ps2", bufs=4, space="PSUM") as ps2p,
            ):
                def conv1(f):
                    xs_f = xs_f_tiles[f]
                    xq = xq_tiles[f]
                    ys = ysp.tile([128, GH, GW], f16, tag="ys")
                    ys_tiles[f] = ys
                    nc.gpsimd.memset(ys[0:64, 1:35, 0:1], 0.0)
                    nc.gpsimd.memset(ys[0:64, 1:35, 129:130], 0.0)
                    for rc in range(9):
                        g0 = 1 + 4 * rc
                        nr = 4 if rc < 8 else 2
                        ps = ps1p.tile([DIM, 4, W], f32, tag="ps1")
                        for dj in range(3):
                            nc.tensor.matmul(
                                ps[:, :nr, :],
                                lhsT=w1s_sb[:, dj, :],
                                rhs=xs_f[:, g0 - 1:g0 - 1 + nr, dj:dj + W],
                                start=(dj == 0), stop=False)
                        if USE_QCONV:
                            nc.tensor.matmul(
                                ps[:, :nr, :], lhsT=w1q_sb,
                                rhs=xq[:, g0 - 1:g0 - 1 + nr, :],
                                start=False, stop=False)
                            nc.tensor.matmul(
                                ps[:, :nr, :], lhsT=w1r_sb[64:128, 2, :],
                                rhs=xs_f[64:128, g0:g0 + nr, 2:2 + W],
                                start=False, stop=True)
                        else:
                            for dj in range(3):
                                nc.tensor.matmul(
                                    ps[:, :nr, :],
                                    lhsT=w1r_sb[64:128, dj, :],
                                    rhs=xs_f[64:128, g0:g0 + nr, dj:dj + W],
                                    start=False, stop=(dj == 2))
                        nc.scalar.activation(
                            ys[0:64, g0:g0 + nr, 1:129], ps[:, :nr],
                            Act.Prelu, bias=b1_sb, scale=1.0, alpha=0.2)
                    # zero y halo rows outside the image (conv2 zero-pad)
                    nc.scalar.activation(ys[0:64, 1:2, 1:129],
                                         ys[0:64, 1:2, 1:129],
                                         Act.Copy, scale=ym_sb[:, 0:1])
                    nc.scalar.activation(ys[0:64, 34:35, 1:129],
                                         ys[0:64, 34:35, 1:129],
                                         Act.Copy, scale=ym_sb[:, 1:2])
                    # stacked row-shifted copy: ys[64+c, g] = ys[c, g+1]
                    # (gpsimd SWDGE queue; split so conv2 chunk 0 starts early)
                    nc.gpsimd.dma_start(out=ys[64:128, 1:18, :],
                                        in_=ys[0:64, 2:19, :])
                    nc.gpsimd.dma_start(out=ys[64:128, 18:34, :],
                                        in_=ys[0:64, 19:35, :])

                def conv2(f):
                    ys = ys_tiles[f]
                    ker_f = kerp.tile([16, SLAB, W], f16, tag="ker")
                    ker_tiles[f] = ker_f
                    for rc in range(8):
                        g0 = 2 + 4 * rc
                        ps2 = ps2p.tile([9, 4, W], f32, tag="ps2")
                        for dj in range(3):
                            nc.tensor.matmul(
                                ps2,
                                lhsT=w2s_sb[:, dj, :],
                                rhs=ys[:, g0 - 1:g0 + 3, dj:dj + W],
                                start=(dj == 0), stop=False)
                        for dj in range(3):
                            nc.tensor.matmul(
                                ps2, lhsT=w2r_sb[64:128, dj, :],
                                rhs=ys[64:128, g0:g0 + 4, dj:dj + W],
                                start=False, stop=(dj == 2))
                        nc.scalar.activation(
                            ker_f[0:9, 4 * rc:4 * rc + 4, :], ps2,
                            Act.Identity, bias=b2_sb, scale=1.0)

                def post(f):
                    # pivot ker (9, r, w) -> (w, r, 9) via three XBAR
                    # transposes: column-shifted ker copies (built on the
                    # gpsimd SWDGE queue, off the serial Q_I DMA queue)
                    # provide the +-1 pixel shifts for the dj=0/2 taps, so
                    # no partition-shift DMAs are needed. The folds only
                    # modify dj=1 slots of kt, which kt_p1/kt_m1 never read.
                    ker_f = ker_tiles[f]
                    ker_b = kerp.tile([16, SLAB, W], f16, tag="kerb")
                    nc.gpsimd.memset(ker_b[0:9, :, 127:128], 0.0)
                    nc.gpsimd.dma_start(out=ker_b[0:9, :, 0:127],
                                        in_=ker_f[0:9, :, 1:128])
                    ker_c = kerp.tile([16, SLAB, W], f16, tag="kerc")
                    nc.gpsimd.memset(ker_c[0:9, :, 0:1], 0.0)
                    nc.gpsimd.dma_start(out=ker_c[0:9, :, 1:128],
                                        in_=ker_f[0:9, :, 0:127])
                    kt1 = kt1p.tile([W, SLAB, 16], f16, tag="kt1")
                    nc.sync.dma_start(out=kt1, in_=ker_f, transpose=True)
                    kt1p1 = kt1p.tile([W, SLAB, 16], f16, tag="kt1p1")
                    nc.sync.dma_start(out=kt1p1, in_=ker_b, transpose=True)
                    kt1m1 = kt1p.tile([W, SLAB, 16], f16, tag="kt1m1")
                    nc.sync.dma_start(out=kt1m1, in_=ker_c, transpose=True)
                    # repack to (w, 9, r): taps outer, rows contiguous
                    nc.scalar.copy(
                        kt[:, 9 * f:9 * f + 9, :],
                        kt1[:, :, 0:9].rearrange("p r k -> p k r"))
                    nc.gpsimd.tensor_copy(
                        kt_p1[:, 9 * f:9 * f + 9, :],
                        kt1p1[:, :, 0:9].rearrange("p r k -> p k r"))
                    nc.gpsimd.tensor_copy(
                        kt_m1[:, 9 * f:9 * f + 9, :],
                        kt1m1[:, :, 0:9].rearrange("p r k -> p k r"))
                    # fold W-edge replicate-pad terms into the dj=1 slot
                    nc.vector.tensor_tensor(ktr[0:1, f, :, 1, :],
                                            ktr[0:1, f, :, 1, :],
                                            ktr[0:1, f, :, 0, :], Alu.add)
                    nc.vector.scalar_tensor_tensor(
                        out=ktr[96:128, f, :, 1, :],
                        in0=ktr[96:128, f, :, 2, :], scalar=em_sb[96:128, :],
                        in1=ktr[96:128, f, :, 1, :],
                        op0=Alu.mult, op1=Alu.add)
                    # dynamic filtering: DVE multiplies, DMA-accum adds
                    ksrc = [kt_p1, kt, kt_m1]
                    for di in range(3):
                        for dj in range(3):
                            kb = ksrc[dj][:, 9 * f + 3 * di + dj, :]\
                                .unsqueeze(1).broadcast_to((W, DIM, SLAB))
                            xt_sl = xt_sb[:, f, :, di:di + SLAB]
                            prod = stp.tile([W, DIM, SLAB], f16,
                                            tag="prod")
                            nc.vector.tensor_tensor(prod, xt_sl, kb,
                                                    Alu.mult)
                            nc.vector.tensor_tensor(accs[dj], accs[dj],
                                                    prod, Alu.add)
                    # u += xt_f (for the normalization term c * S)
                    nc.vector.tensor_tensor(u_sb, u_sb, xt_sb[:, f],
                                            Alu.add)

                conv1(0)
                conv2(0)
                load_xs(2)
                load_xq(2)
                post(0)
                conv1(1)
                conv2(1)
                load_xs(3)
                load_xq(3)
                post(1)
                conv1(2)
                conv2(2)
                load_xs(4)
                load_xq(4)
                post(2)
                conv1(3)
                conv2(3)
                post(3)
                conv1(4)
                conv2(4)
                post(4)

            # normalization: out += c * S with c = 1/45 - mean(ker);
            # sum45 reads the folded kernel, undo the edge double-count
            sum45 = ktp.tile([W, SLAB], f16)
            kt_v = kt[:, 0:45, :].rearrange("p (t n) r -> p r t n", t=T)
            nc.vector.tensor_reduce(sum45, kt_v, axis=mybir.AxisListType.XY,
                                    op=Alu.add)
            c_sb = ktp.tile([W, SLAB], f16)
            nc.vector.tensor_scalar(c_sb, sum45, -1.0 / 45.0, 1.0 / 45.0,
                                    Alu.mult, Alu.add)
            corr = ktp.tile([W, SLAB], f16)
            kt_e = kt[:, 0:45, :].rearrange(
                "p (t di dj) r -> p r t di dj", t=T, di=3, dj=3)
            nc.vector.tensor_reduce(corr[0:32], kt_e[0:32, :, :, :, 0],
                                    axis=mybir.AxisListType.XY, op=Alu.add)
            nc.vector.tensor_reduce(corr[96:128], kt_e[96:128, :, :, :, 2],
                                    axis=mybir.AxisListType.XY, op=Alu.add)
            nc.vector.scalar_tensor_tensor(out=c_sb[0:32], in0=corr[0:32],
                                           scalar=ea_sb[0:32], in1=c_sb[0:32],
                                           op0=Alu.mult, op1=Alu.add)
            nc.vector.scalar_tensor_tensor(out=c_sb[96:128], in0=corr[96:128],
                                           scalar=eb_sb[96:128],
                                           in1=c_sb[96:128],
                                           op0=Alu.mult, op1=Alu.add)

            # S = 3-row vertical box of u (edge rows already clamped in xt)
            s_sb = accp.tile([W, DIM, SLAB], f16)
            nc.vector.tensor_tensor(s_sb, u_sb[:, :, 0:SLAB],
                                    u_sb[:, :, 1:SLAB + 1], Alu.add)
            nc.vector.tensor_tensor(s_sb, s_sb, u_sb[:, :, 2:SLAB + 2],
                                    Alu.add)

            # shifted + edge-doubled variants of c
            c_p1 = ktp.tile([W, SLAB], f16)
            c_m1 = ktp.tile([W, SLAB], f16)
            nc.gpsimd.memset(c_p1[96:128], 0.0)
            nc.gpsimd.memset(c_m1[0:32], 0.0)
            nc.sync.dma_start(out=c_p1[0:127], in_=c_sb[1:128])
            nc.sync.dma_start(out=c_m1[1:128], in_=c_sb[0:127])
            c_c = ktp.tile([W, SLAB], f16)
            nc.vector.tensor_scalar(c_c, c_sb, ef_sb, None, Alu.mult)
            for dj, csrc in ((0, c_p1), (1, c_c), (2, c_m1)):
                cb = csrc.unsqueeze(1).broadcast_to((W, DIM, SLAB))
                prod = stp.tile([W, DIM, SLAB], f16, tag="prod")
                nc.vector.tensor_tensor(prod, s_sb, cb, Alu.mult)
                nc.vector.tensor_tensor(accs[dj], accs[dj], prod, Alu.add)

            # merge after transposition: XBAR each acc to [m, o, w] where
            # the dj pixel shift is a free-dim (w) offset
            obt = []
            for dj in range(3):
                t = accp.tile([128, 16, 128], f16, name=f"obt{dj}")
                nc.sync.dma_start(
                    out=t,
                    in_=accs[dj].rearrange("p (o a) r -> p o (a r)",
                                           o=16, a=4),
                    transpose=True)
                obt.append(t)
            # out[., ., w] = T1[w] + T0[w-1] + T2[w+1]
            nc.vector.tensor_tensor(obt[1][:, :, 1:128], obt[1][:, :, 1:128],
                                    obt[0][:, :, 0:127], Alu.add)
            nc.vector.tensor_tensor(obt[1][:, :, 0:127], obt[1][:, :, 0:127],
                                    obt[2][:, :, 1:128], Alu.add)
            # obt1[m, o, w]: c = 4o + m//32, r = m%32
            out_v = out_d.rearrange("(o c4) r w -> (c4 r) o w", o=16, c4=4)
            nc.sync.dma_start(out=out_v, in_=obt[1])

    return nc


def _get_program():
    if "nc" not in _PROGRAM_CACHE:
        nc = _build_program()
        nc.finalize()
        _PROGRAM_CACHE["nc"] = nc
    return _PROGRAM_CACHE["nc"]


def _host_prep(x, w1, b1, w2, b2):
    """Build the 8 per-core input maps from full inputs."""
    x = np.asarray(x, dtype=np.float32)
    w1 = np.asarray(w1, dtype=np.float32)
    b1 = np.asarray(b1, dtype=np.float32)
    w2 = np.asarray(w2, dtype=np.float32)
    b2 = np.asarray(b2, dtype=np.float32)

    # stacked conv weights: row pairs di=a in partition halves; col pair
    # (di=2, dj=a); leftover (di=2, dj=2) in partitions 64..127
    w1s = np.zeros((128, 3, DIM), dtype=np.float16)
    w1r = np.zeros((128, 3, DIM), dtype=np.float16)
    w1q = np.zeros((128, DIM), dtype=np.float16)
    for a in range(2):
        w1s[64 * a:64 * a + 64] = w1[:, :, a, :].transpose(1, 2, 0)
        w1q[64 * a:64 * a + 64] = w1[:, :, 2, a].transpose(1, 0)
    w1r[64:128] = w1[:, :, 2, :].transpose(1, 2, 0)
    w2s = np.zeros((128, 3, 9), dtype=np.float16)
    w2r = np.zeros((128, 3, 9), dtype=np.float16)
    for a in range(2):
        w2s[64 * a:64 * a + 64] = w2[:, :, a, :].transpose(1, 2, 0)
    w2r[64:128] = w2[:, :, 2, :].transpose(1, 2, 0)

    b1c = np.ascontiguousarray(b1.reshape(DIM, 1))
    b2c = np.ascontiguousarray(b2.reshape(9, 1))
    emask = np.zeros((W, 1), dtype=np.float32)
    emask[127, 0] = 1.0
    efold = np.ones((W, 1), dtype=np.float32)
    efold[0, 0] = 2.0
    efold[127, 0] = 2.0
    emA = np.zeros((W, 1), dtype=np.float32)
    emA[0, 0] = 1.0 / 45.0
    emB = np.zeros((W, 1), dtype=np.float32)
    emB[127, 0] = 1.0 / 45.0

    x16 = x.astype(np.float16)
    in_maps = []
    for core in range(NCORES):
        b, s = divmod(core, 4)
        r0 = s * SLAB
        # conv input, stacked: xs[c+64a, f, g, w] = xpad[c, f, r0-2+g+a, w-1]
        xs = np.zeros((128, T, GH, GW), dtype=np.float16)
        for a in range(2):
            lo = r0 - 2 + a
            hi = lo + GH            # rows lo .. hi-1
            clo = max(0, lo)
            chi = min(H, hi)
            if chi > clo:
                xs[64 * a:64 * a + 64, :, clo - lo:chi - lo, 1:129] = \
                    x16[b, :, :, clo:chi, :]
        # filter input, pixel-partitioned: xt[w, f, c, r]
        rows = np.clip(np.arange(r0 - 1, r0 + 33), 0, H - 1)
        xt = np.ascontiguousarray(
            x16[b][:, :, rows, :].transpose(3, 1, 0, 2))
        # col-pair-stacked conv1 input: xq[c+64a, f, g, w] =
        #   xpad[c, f, r0+g, w-1+a]   (g = 0..33)
        xq = np.zeros((128, T, GH - 2, W), dtype=np.float16)
        for a in range(2):
            lo = r0
            hi = lo + GH - 2
            clo = max(0, lo)
            chi = min(H, hi)
            if chi > clo:
                if a == 0:
                    xq[0:64, :, clo - lo:chi - lo, 1:128] = \
                        x16[b, :, :, clo:chi, 0:127]
                else:
                    xq[64:128, :, clo - lo:chi - lo, 0:128] = \
                        x16[b, :, :, clo:chi, 0:128]
        ymask = np.ones((DIM, 2), dtype=np.float32)
        if s == 0:
            ymask[:, 0] = 0.0
        if s == 3:
            ymask[:, 1] = 0.0
        in_maps.append({
            "xs": xs, "xt": xt, "xq": xq, "w1s": w1s, "w1r": w1r, "w1q": w1q,
            "w2s": w2s, "w2r": w2r, "b1c": b1c, "b2c": b2c,
            "ymask": ymask, "emask": emask, "efold": efold, "emA": emA,
            "emB": emB,
        })
    return in_maps


def kernel(x, w1, b1, w2, b2):
    from concourse.bass_utils import run_bass_kernel_spmd

    nc = _get_program()
    in_maps = _host_prep(x, w1, b1, w2, b2)
    res = run_bass_kernel_spmd(nc, in_maps, list(range(NCORES)))
    out = np.zeros((2, DIM, H, W), dtype=np.float32)
    for core in range(NCORES):
        b, s = divmod(core, 4)
        out[b, :, s * SLAB:(s + 1) * SLAB, :] = \
            res.results[core]["out"].astype(np.float32)
    return out
